# revision 2
# baseline (speedup 1.0000x reference)
"""Trainium2 Bass kernel for nn_ContrastiveLoss (SCAN t2i contrastive loss).

Caption-sharded across 8 cores: each core holds all B=128 images and a
16-caption slice, computes scores[:, c_shard] on device; host gathers the
(B, B) score matrix and applies the tiny hinge loss.

Per (image, caption) pair the reference computes leaky+l2-normalized
attention over regions, softmax, a weighted image context per word, and
cosine similarities.  Gram identity avoids the (W, D) weighted context:
    P1[w] = sum_r E*G,   P2[w] = E^T Mi E,   E = exp(9*An)
    row_sim = P1 / (w1 * sqrt(P2))        (softmax normalizer S cancels)
with G = im_i @ cap_c^T and Mi = im_i @ im_i^T (block-diag, host-computed).

Key perf structure vs the earlier version:
  - single streaming pass over 43 image chunks (42 triples + 1 pair =
    exactly 128 accumulator rows), both caption halves fused inside, so
    imt tiles are transient and DMA fully overlaps compute
  - rsqrt as exp(-0.5*ln(x)) so the scalar engine only ever uses the
    natural_log_exp activation-table set (no ACT_TABLE_LOAD thrash)
  - Mi Gram blocks precomputed on host (removes LDW-bound PE matmuls)
  - bf16 elementwise tiles for DVE 2x mode; an-multiply on GpSimd
"""

import json

import numpy as np
import ml_dtypes

import concourse.bass as bass
import concourse.mybir as mybir
import concourse.tile as tile
from concourse.bass_utils import run_bass_kernel_spmd


def _split_waits(bir_bytes, maxw=1):
    """Walrus in this toolchain accepts only `maxw` sync-waits per
    instruction; hoist extras onto preceding 1-wait Drain no-ops."""
    bir = json.loads(bir_bytes)
    for fn in bir["functions"]:
        for blk in fn["blocks"]:
            out = []
            for inst in blk["instructions"]:
                si = inst.get("sync_info") or {}
                ow = si.get("on_wait") or []
                if len(ow) > maxw:
                    head, tail = ow[:-maxw], ow[-maxw:]
                    for j, w in enumerate(head):
                        out.append({"debug": inst.get("debug"),
                                    "engine": inst["engine"], "ins": [],
                                    "is_reset_sema": False,
                                    "name": f"{inst['name']}-w{j}",
                                    "opcode": "Drain", "outs": [],
                                    "sync_info": {"on_update": [],
                                                  "on_wait": [w]}})
                    si["on_wait"] = tail
                out.append(inst)
            blk["instructions"] = out
    return json.dumps(bir).encode()


F32 = mybir.dt.float32
BF16 = mybir.dt.bfloat16
FP8 = mybir.dt.float8e4
AF = mybir.ActivationFunctionType
ALU = mybir.AluOpType

LAMBDA_SOFTMAX = 9.0
LAMBDA_LSE = 6.0
MARGIN = 0.2

B, R, W, D = 128, 36, 50, 1024
NCORES = 8
CS = B // NCORES            # 16 captions per core
KD = D // 128               # 8 contraction chunks
KQ = D // 256               # 4 fp8 DoubleRow chunks (K=256 each)
WD = CS * W                 # 800 words per core
NHALF = 2
HW_ = WD // NHALF           # 400 free elements per half
CH = CS // NHALF            # 8 captions per half
NT = 43                     # 42 triples of 3 images + 1 pair of 2
PT = 3 * R                  # 108 region partitions per triple
PTP = 112                   # PT padded so the fp8 DoubleRow k-stride %16==0
PP = 2 * R                  # 72 region partitions of the final pair
DELTA = 1e-12


def _build_nc():
    nc = bass.Bass("TRN2", target_bir_lowering=False, debug=False,
                   num_devices=NCORES)

    imT = nc.dram_tensor("imT", [128, NT, KQ, 2, PTP], FP8, kind="ExternalInput")
    capT = nc.dram_tensor("capT", [128, KQ, 2, WD], FP8, kind="ExternalInput")
    msbT = nc.dram_tensor("msbT", [PT, NT * PT], BF16, kind="ExternalInput")
    onesT = nc.dram_tensor("onesT", [PT, 2 * B], BF16, kind="ExternalInput")
    onesP = nc.dram_tensor("onesP", [PP, B], BF16, kind="ExternalInput")
    w1r_row = nc.dram_tensor("w1r", [WD], BF16, kind="ExternalInput")
    mask_row = nc.dram_tensor("maskr", [WD], BF16, kind="ExternalInput")
    scores_d = nc.dram_tensor("scores", [B, CS], F32, kind="ExternalOutput")

    with tile.TileContext(nc) as tc:
        with (
            tc.tile_pool(name="const", bufs=1) as const,
            tc.tile_pool(name="imt", bufs=3) as imtp,
            tc.tile_pool(name="work", bufs=6) as work,
            tc.tile_pool(name="small", bufs=6) as small,
            tc.tile_pool(name="drain", bufs=1) as drain,
            tc.tile_pool(name="pg", bufs=2, space="PSUM") as pg,
            tc.tile_pool(name="pu", bufs=2, space="PSUM") as pu,
            tc.tile_pool(name="pacc", bufs=1, space="PSUM") as pacc,
        ):
            # ---- resident constants ----
            cap_sb = const.tile([128, KQ, 2, WD], FP8)
            nc.gpsimd.dma_start(out=cap_sb, in_=capT.ap())
            msb_sb = const.tile([PT, NT * PT], BF16)
            nc.gpsimd.dma_start(out=msb_sb, in_=msbT.ap())
            ones_sb = const.tile([PT, 2 * B], BF16)
            nc.gpsimd.dma_start(out=ones_sb, in_=onesT.ap())
            onesp_sb = const.tile([PP, B], BF16)
            nc.gpsimd.dma_start(out=onesp_sb, in_=onesP.ap())
            w1rb = const.tile([B, WD], BF16)
            nc.gpsimd.dma_start(out=w1rb,
                                in_=w1r_row.ap()[None, :].to_broadcast([B, WD]))
            mkb = const.tile([B, WD], BF16)
            nc.gpsimd.dma_start(out=mkb,
                                in_=mask_row.ap()[None, :].to_broadcast([B, WD]))
            eps_c = const.tile([B, 1], F32)
            nc.vector.memset(eps_c, DELTA)

            accs = [(pacc.tile([B, HW_], F32, tag=f"p1h{h}", name=f"p1h{h}"),
                     pacc.tile([B, HW_], F32, tag=f"p2h{h}", name=f"p2h{h}"))
                    for h in range(NHALF)]

            def acc_section(ctx):
                pt, ones_l, msb, e_t, prod1, mm_flags = ctx
                for h in range(NHALF):
                    p1a, p2a = accs[h]
                    wsl = slice(h * HW_, (h + 1) * HW_)
                    nc.tensor.matmul(p1a, lhsT=ones_l, rhs=prod1[:pt, wsl],
                                     **mm_flags)
                    ups = pu.tile([PT, HW_], F32, tag="u", name="u")
                    nc.tensor.matmul(ups[:pt], lhsT=msb, rhs=e_t[:pt, wsl],
                                     start=True, stop=True)
                    prod2 = work.tile([PT, HW_], BF16, tag=f"prod2{h}",
                                      name=f"prod2{h}")
                    nc.scalar.activation(out=prod2[:pt], in_=ups[:pt],
                                         func=AF.Square)
                    nc.tensor.matmul(p2a, lhsT=ones_l, rhs=prod2[:pt], **mm_flags)

            pending = None
            for t in range(NT):
                pt = PP if t == NT - 1 else PT
                imt_f = imtp.tile([128, KQ, 2, PTP], FP8, tag="imt")
                nc.sync.dma_start(out=imt_f, in_=imT.ap()[:, t])
                imt = imt_f[:, :, :, :pt]
                if t == NT - 1:
                    ones_l = onesp_sb
                    msb = msb_sb[:pt, t * PT:t * PT + pt]
                else:
                    ones_l = ones_sb[:, B - 3 * t:2 * B - 3 * t]
                    msb = msb_sb[:, t * PT:(t + 1) * PT]
                mm_flags = dict(start=(t == 0), stop=(t == NT - 1),
                                skip_group_check=True)

                # G matmuls per half (PSUM bank width), fused elementwise
                graw = work.tile([PT, WD], BF16, tag="graw", name="graw")
                gps = []
                for h in range(NHALF):
                    g = pg.tile([PT, HW_], F32, tag="G")
                    for k in range(KQ):
                        nc.tensor.matmul(
                            g[:pt], lhsT=imt[:, k],
                            rhs=cap_sb[:, k, :, h * HW_:(h + 1) * HW_],
                            start=(k == 0), stop=(k == KQ - 1),
                            perf_mode=mybir.MatmulPerfMode.DoubleRow)
                    gps.append(g)
                    if h == 0:
                        nc.scalar.copy(out=graw[:pt, h * HW_:(h + 1) * HW_],
                                       in_=g[:pt])
                    else:
                        nc.vector.tensor_copy(out=graw[:pt, h * HW_:(h + 1) * HW_],
                                              in_=g[:pt])

                # word chain on (pt, 800) tiles
                a_t = work.tile([PT, WD], BF16, tag="A", name="A")
                nc.vector.scalar_tensor_tensor(
                    out=a_t[:pt], in0=graw[:pt], scalar=0.1, in1=graw[:pt],
                    op0=ALU.mult, op1=ALU.max)
                sq = work.tile([PT, WD], BF16, tag="sq", name="sq")
                nc.gpsimd.tensor_tensor(out=sq[:pt], in0=a_t[:pt],
                                        in1=a_t[:pt], op=ALU.mult)
                nrm = small.tile([PT, CS], F32, tag="nrm", name="nrm")
                nc.vector.tensor_reduce(
                    out=nrm[:pt], in_=sq[:pt].rearrange("p (c w) -> p c w", w=W),
                    axis=mybir.AxisListType.X, op=ALU.add)
                lnn = small.tile([PT, CS], F32, tag="lnn", name="lnn")
                nc.scalar.activation(out=lnn[:pt], in_=nrm[:pt], func=AF.Ln,
                                     bias=eps_c[:pt])
                rcp = small.tile([PT, CS], BF16, tag="rcp", name="rcp")
                nc.scalar.activation(out=rcp[:pt], in_=lnn[:pt], func=AF.Exp,
                                     scale=-0.5)
                an = work.tile([PT, WD], BF16, tag="an", name="an")
                nc.gpsimd.tensor_tensor(
                    out=an[:pt].rearrange("p (c w) -> p c w", w=W),
                    in0=a_t[:pt].rearrange("p (c w) -> p c w", w=W),
                    in1=rcp[:pt, :, None].to_broadcast([pt, CS, W]),
                    op=ALU.mult)
                e_t = work.tile([PT, WD], BF16, tag="E", name="E")
                nc.scalar.activation(out=e_t[:pt], in_=an[:pt], func=AF.Exp,
                                     scale=LAMBDA_SOFTMAX)
                prod1 = work.tile([PT, WD], BF16, tag="prod1", name="prod1")
                nc.vector.tensor_tensor(out=prod1[:pt], in0=e_t[:pt],
                                        in1=graw[:pt], op=ALU.mult)

                # acc matmuls for the PREVIOUS chunk (1-chunk software
                # pipeline: keeps the next G burst ahead of acc matmuls
                # that wait on this chunk's elementwise chain)
                if pending is not None:
                    acc_section(pending)
                pending = (pt, ones_l, msb, e_t, prod1, mm_flags)
            acc_section(pending)

            # ---- drain: per-word math on (B, HW_) tiles ----
            for h in range(NHALF):
                p1a, p2a = accs[h]
                wsl = slice(h * HW_, (h + 1) * HW_)
                l2 = drain.tile([B, HW_], F32, tag="l2")
                nc.scalar.activation(out=l2, in_=p2a, func=AF.Ln, bias=eps_c)
                rcp2 = drain.tile([B, HW_], BF16, tag="rcp2")
                nc.scalar.activation(out=rcp2, in_=l2, func=AF.Exp, scale=-0.5)
                rs1 = drain.tile([B, HW_], BF16, tag="rs1")
                nc.vector.tensor_tensor(out=rs1, in0=p1a, in1=rcp2, op=ALU.mult)
                rs = drain.tile([B, HW_], BF16, tag="rs")
                nc.vector.tensor_tensor(out=rs, in0=rs1, in1=w1rb[:, wsl],
                                        op=ALU.mult)
                xx = drain.tile([B, HW_], BF16, tag="xx")
                nc.scalar.activation(out=xx, in_=rs, func=AF.Exp,
                                     scale=LAMBDA_LSE)
                xm = drain.tile([B, HW_], BF16, tag="xm")
                nc.vector.tensor_tensor(out=xm, in0=xx, in1=mkb[:, wsl],
                                        op=ALU.mult)
                lse = drain.tile([B, CH], F32, tag="lse")
                nc.vector.tensor_reduce(
                    out=lse, in_=xm.rearrange("p (c w) -> p c w", w=W),
                    axis=mybir.AxisListType.X, op=ALU.add)
                sc = drain.tile([B, CH], F32, tag="sc")
                nc.scalar.activation(out=sc, in_=lse, func=AF.Ln)
                nc.vector.tensor_scalar_mul(out=sc, in0=sc,
                                            scalar1=1.0 / LAMBDA_LSE)
                nc.sync.dma_start(out=scores_d.ap()[:, h * CH:(h + 1) * CH],
                                  in_=sc)

    _orig = nc.to_json_bytes
    nc.to_json_bytes = lambda *a, **k: _split_waits(_orig(*a, **k))
    return nc


_NC = None
# test-harness hooks (harmless defaults for grading)
TRACE = False
LAST_RESULTS = None


def _bf16(x):
    return np.ascontiguousarray(np.asarray(x, dtype=ml_dtypes.bfloat16))


def _fp8(x):
    return np.ascontiguousarray(np.asarray(x, dtype=ml_dtypes.float8_e4m3))


def _host_prep(im, s, s_l):
    im = np.ascontiguousarray(np.asarray(im, np.float32))
    s = np.asarray(s, np.float32)
    s_l = np.asarray(s_l)
    mask = (np.arange(W)[None, :] < s_l[:, None]).astype(np.float32)
    cap = np.ascontiguousarray(s * mask[:, :, None])
    w1 = np.sqrt(np.einsum('cwd,cwd->cw', cap, cap, dtype=np.float32,
                           optimize=True))
    w1r = (mask / np.maximum(w1, 1e-20)).reshape(B * W)

    # imT: chunk-major, fp8 DoubleRow layout — feature f = k*256 + j*128 + p.
    imf = im.reshape(B * R, D)
    imT = np.zeros((128, NT, KQ, 2, PTP), np.float32)
    src = imf.T.reshape(KQ, 2, 128, B * R).transpose(2, 0, 1, 3)  # (128,KQ,2,BR)
    for t in range(NT):
        pt = PP if t == NT - 1 else PT
        imT[:, t, :, :, :pt] = src[:, :, :, t * PT:t * PT + pt]
    imT = _fp8(imT)

    # Host Cholesky factors: Mi = L L^T, so P2 = sum_q (L^T E)_q^2 and the
    # per-word product becomes a pure Square (scalar engine, from PSUM).
    g3 = np.matmul(im, im.transpose(0, 2, 1))  # (B, R, R)
    L = np.linalg.cholesky(g3 + 1e-4 * np.eye(R, dtype=np.float32)[None])
    msbT = np.zeros((PT, NT * PT), np.float32)
    for i in range(B):
        t, j = divmod(i, 3)
        msbT[j * R:(j + 1) * R, t * PT + j * R:t * PT + (j + 1) * R] = L[i]
    msbT = _bf16(msbT)

    onesT = np.zeros((PT, 2 * B), np.float32)
    for j in range(3):
        onesT[j * R:(j + 1) * R, B + j] = 1.0
    onesP = np.zeros((PP, B), np.float32)
    for j in range(2):
        onesP[j * R:(j + 1) * R, 126 + j] = 1.0
    onesT, onesP = _bf16(onesT), _bf16(onesP)

    in_maps = []
    for c in range(NCORES):
        c0 = c * CS
        capf = cap[c0:c0 + CS].reshape(WD, D)
        capT = _fp8(capf.T.reshape(KQ, 2, 128, WD).transpose(2, 0, 1, 3))
        in_maps.append({
            "imT": imT,
            "capT": capT,
            "msbT": msbT,
            "onesT": onesT,
            "onesP": onesP,
            "w1r": _bf16(w1r[c0 * W:(c0 + CS) * W]),
            "maskr": _bf16(mask[c0:c0 + CS].reshape(WD)),
        })
    return in_maps


def kernel(im, im_l, s, s_l):
    global _NC, LAST_RESULTS
    if _NC is None:
        _NC = _build_nc()
    in_maps = _host_prep(im, s, s_l)
    res = run_bass_kernel_spmd(_NC, in_maps, core_ids=list(range(NCORES)),
                               trace=TRACE)
    LAST_RESULTS = res
    scores = np.concatenate([r["scores"] for r in res.results], axis=1)

    diag = np.diagonal(scores)[:, None]
    cost_s = np.maximum(MARGIN + scores - diag, 0.0)
    cost_im = np.maximum(MARGIN + scores - diag.T, 0.0)
    np.fill_diagonal(cost_s, 0.0)
    np.fill_diagonal(cost_im, 0.0)
    loss = np.sum(np.max(cost_s, axis=1)) + np.sum(np.max(cost_im, axis=0))
    return np.array(loss, np.float32)
